# revision 27
# baseline (speedup 1.0000x reference)
"""VisionZip text-aware token-selection kernel for Trainium2 (Bass/Tile), v5.

Contract: kernel(**inputs) takes FULL inputs (B=32) and returns the FULL
output [32, 65, 1024]. Pure data-parallel over 8 NeuronCores (4 samples each).

Key design points (vs the fp16 hi/lo baseline):
  - hidden as a single bf16 copy: the PE runs fp16 at 2 cycles/row but bf16
    at 1, so one bf16 pass is ~4x faster than the hi/lo fp16 pair; hidden
    HBM traffic is halved. Total output error ~3e-3 << the 2e-2 gate.
  - 1/cnt folded into the C matrix (C_ctx = itgt + eqm/cnt): PSUM result is
    final, no per-output recip multiply.
  - affine score trick: rank order of 0.5*z(Sd)+0.5*z(cos) equals rank order
    of a*Sd + b*cos with a=0.5/(std+eps) per sample (means drop out), so no
    mean-subtraction passes; stats via ones-matmuls on the column layout.
  - column-major layout [token(128p), chunk, sample] everywhere; attn CLS row
    and metric host-transposed so Sd is one X-reduce.
  - rank split across Vector (fused is_gt+accum) and Scalar/ACT (fused
    Sign+accum with per-partition bias; #gt = (sum+576)/2 folded into the
    mask threshold).
  - metric normalization never materialized: sim rows may be scaled freely
    (argmax over targets is invariant), so sim uses raw metric; rnorm is
    folded into the Tn right-hand side (itgt_w = itgt * rnorm).
  - Tn/sim/transposes in float32r (fast fp32 mode); sim batched over sample
    pairs with block-diagonal Tn (one N=20 matmul per chunk pair).
"""
import numpy as np

import sys
if '/opt/trn_rl_repo' not in sys.path:
    sys.path.insert(0, '/opt/trn_rl_repo')

import concourse.bacc as bacc
import concourse.tile as tile
from concourse import mybir
from concourse.bass_utils import run_bass_kernel_spmd

F32 = mybir.dt.float32
F32R = mybir.dt.float32r
BF16 = mybir.dt.bfloat16
NPBF16 = mybir.dt.np(mybir.dt.bfloat16)
N_CORES = 8
BC = 4                      # samples per core
L = 577                     # tokens (incl CLS)
LPAD = 640
D = 1024
CK = 64
NH = 16
DOM = 54
NSEL = DOM + 1              # + CLS
CTX = 10
STEP = 52                   # (577-1-54) // 10
OUT_T = NSEL + CTX          # 65 output tokens
CHUNKS = [(0, 128), (128, 128), (256, 128), (384, 128), (512, 65)]
EQ = mybir.AluOpType
AF = mybir.ActivationFunctionType
AX = mybir.AxisListType
# rank engine split: (ci, s) -> 'V' or 'A'
RANK_ENG = {(ci, s): ('V' if (ci < 2 or (ci == 2 and s < 2)) else 'A')
            for ci in range(5) for s in range(BC)}


def _consts():
    c = {}
    c["c_iden"] = np.eye(128, dtype=np.float32)
    ut = (np.arange(128)[:, None] <= np.arange(128)[None, :]).astype(NPBF16)
    c["c_utb"] = ut
    c["c_onesb"] = np.ones((128, 128), NPBF16)
    c["c_onescol"] = np.ones((128, 1), np.float32)
    c["c_ones1"] = np.ones((1, 128), np.float32)
    selbc = np.zeros((BC, BC * 128), np.float32)
    for s in range(BC):
        selbc[s, s * 128:(s + 1) * 128] = 1.0
    c["c_selbc"] = selbc
    c["c_iota55"] = (np.arange(NSEL) + 1.0).astype(np.float32).reshape(1, 1, NSEL) \
        .repeat(128, 0).copy()
    c["c_iota52"] = (-STEP * np.arange(CTX, dtype=np.float32)).reshape(1, 1, CTX) \
        .repeat(128, 0).copy()
    ii = np.zeros((128, 5, 1), np.float32)
    for ci, (off, _) in enumerate(CHUNKS):
        ii[:, ci, 0] = off + np.arange(128)
    c["c_iotaI"] = ii
    return c


_CONST_DTYPES = {"c_utb": BF16, "c_onesb": BF16}


def build_nc(stage=99):
    nc = bacc.Bacc("TRN2", target_bir_lowering=False, debug=False)

    metricT_d = nc.declare_dram_parameter("metricT", [128, 5, BC, CK], F32, isOutput=False)
    attnT_d = nc.declare_dram_parameter("attnT", [128, 5, BC * NH], F32, isOutput=False)
    text_d = nc.declare_dram_parameter("text", [BC, CK], F32, isOutput=False)
    hid_d = nc.declare_dram_parameter("hidb", [BC, L, D], BF16, isOutput=False)
    cshapes = {k: v.shape for k, v in _consts().items()}
    cdram = {k: nc.declare_dram_parameter(k, list(sh), _CONST_DTYPES.get(k, F32),
                                          isOutput=False)
             for k, sh in cshapes.items()}
    out_d = nc.declare_dram_parameter("out", [BC, OUT_T, D], F32, isOutput=True)

    with tile.TileContext(nc) as tc:
        with (
            tc.tile_pool(name="persist", bufs=1) as pp,
            tc.tile_pool(name="hidpool", bufs=1) as hp,
            tc.tile_pool(name="scratch", bufs=2) as sp,
            tc.tile_pool(name="ps_misc", bufs=2, space="PSUM") as ps_misc,
            tc.tile_pool(name="ps_bcs", bufs=2, space="PSUM") as ps_bcs,
            tc.tile_pool(name="ps_out", bufs=2, space="PSUM") as ps_out,
        ):
            pools = (pp, hp, sp, ps_misc, ps_bcs, ps_out)
            _body(nc, stage, pools, metricT_d, attnT_d, text_d, hid_d,
                  cdram, cshapes, out_d)
    nc.compile()
    return nc


def _body(nc, stage, pools, metricT_d, attnT_d, text_d, hid_d,
          cdram, cshapes, out_d):
    pp, hp, sp, ps_misc, ps_bcs, ps_out = pools
    V = nc.vector
    A = nc.scalar
    G = nc.gpsimd
    T = nc.tensor
    DMA = nc.sync          # inputs: sync-engine HWDGE ring
    DMA2 = nc.scalar       # outputs: scalar-engine HWDGE ring

    def dump(n):
        d = sp.tile([BC, 512], F32, tag="dump")
        V.memset(d[:], float(n))
        DMA.dma_start(out_d[:, 0, 0:512], d[:])

    # ---- input DMAs (text+metric first: they gate the longest chain) ----
    text_sb = pp.tile([BC, CK], F32, tag="text_sb")
    DMA.dma_start(text_sb[:], text_d[:])
    mt = pp.tile([128, 5, BC, CK], F32, tag="mt")
    DMA.dma_start(mt[:], metricT_d[:])
    attnT = pp.tile([128, 5, BC * NH], F32, tag="attnT")
    DMA.dma_start(attnT[:], attnT_d[:])
    csb = {}
    for k, sh in cshapes.items():
        t = pp.tile(list(sh), _CONST_DTYPES.get(k, F32), tag=k)
        DMA.dma_start(t[:], cdram[k][:])
        csb[k] = t
    hid = []
    for s in range(BC):
        row = []
        for ci, (off, k) in enumerate(CHUNKS):
            th = hp.tile([128, D], BF16, tag=f"h{s}_{ci}")
            DMA.dma_start(th[0:k, :], hid_d[s, off:off + k, :])
            row.append(th)
        hid.append(row)
    # prefetch ACT tables during DMA wait (Sqrt + Sign)
    dmt = sp.tile([1, 2], F32, tag="dmt")
    V.memset(dmt[:], 1.0)
    dmt2 = sp.tile([1, 2], F32, tag="dmt2")
    A.activation(dmt2[:, 0:1], dmt[:, 0:1], AF.Sqrt)
    A.activation(dmt2[:, 1:2], dmt[:, 1:2], AF.Sign)

    if stage <= 1:
        return dump(1)

    # ---- text_n and its partition-broadcast ----
    tsc = sp.tile([BC, CK], F32, tag="tsc")
    V.tensor_mul(tsc[:], text_sb[:], text_sb[:])
    tss = pp.tile([BC, 1], F32, tag="tss")
    V.tensor_reduce(tss[:], tsc[:], axis=AX.X, op=EQ.add)
    tst = pp.tile([BC, 1], F32, tag="tst")
    A.activation(tst[:], tss[:], AF.Sqrt)
    trc = pp.tile([BC, 1], F32, tag="trc")
    V.reciprocal(trc[:], tst[:])
    textn = pp.tile([BC, CK], F32, tag="textn")
    V.tensor_scalar_mul(textn[:], text_sb[:], trc[:])
    tb_ps = ps_misc.tile([128, BC * CK], F32, tag="ps")
    for s in range(BC):
        T.matmul(tb_ps[:, s * CK:(s + 1) * CK],
                 csb["c_selbc"][:, s * 128:(s + 1) * 128],
                 textn[:, :], start=True, stop=True)
    textb = pp.tile([128, BC, CK], F32, tag="textb")
    A.copy(textb[:].rearrange("p s c -> p (s c)"), tb_ps[:, :])

    # ---- X tile: [128, 5, (sd s0..3 | cos s0..3 | sd^2 | cos^2)] ----
    X = pp.tile([128, 5, 16], F32, tag="X")
    V.tensor_reduce(X[:, :, 0:4], attnT[:].rearrange("p c (s h) -> p c s h", h=NH),
                    axis=AX.X, op=EQ.add)

    # ---- metric: ssq (square on ACT), rnorm, cos; no normalized copy ----
    nrm = pp.tile([128, 5, BC, 1], F32, tag="nrm")
    for ci, (off, k) in enumerate(CHUNKS):
        sq = sp.tile([128, BC, CK], F32, tag="sq")
        A.activation(sq[0:k].rearrange("p s c -> p (s c)"),
                     mt[0:k, ci].rearrange("p s c -> p (s c)"), AF.Square)
        ssq = sp.tile([128, BC], F32, tag="ssq")
        V.tensor_reduce(ssq[0:k], sq[0:k], axis=AX.X, op=EQ.add)
        srt = sp.tile([128, BC], F32, tag="srt")
        A.activation(srt[0:k], ssq[0:k], AF.Sqrt)
        V.reciprocal(nrm[0:k, ci, :, 0], srt[0:k])
        dq = sp.tile([128, BC, CK], F32, tag="dq")
        V.tensor_mul(dq[0:k], mt[0:k, ci], textb[0:k])
        dsum = sp.tile([128, BC], F32, tag="dsum")
        V.tensor_reduce(dsum[0:k], dq[0:k], axis=AX.X, op=EQ.add)
        V.tensor_mul(X[0:k, ci, 4:8], dsum[0:k], nrm[0:k, ci, :, 0])

    # CLS excluded from z-stats
    V.memset(X[0:1, 0, 0:8], 0.0)
    V.tensor_mul(X[:, :, 8:16], X[:, :, 0:8], X[:, :, 0:8])

    # ---- mtT (pair-stacked transposes of raw metric) ----
    mtT2 = []
    for p in range(2):
        t = pp.tile([128, LPAD], F32, tag=f"mtT{p}")
        for ci, (off, k) in enumerate(CHUNKS):
            tps = ps_misc.tile([128, 128], F32, tag="ps")
            T.transpose(tps[:, :],
                        mt[:, ci, 2 * p:2 * p + 2, :].rearrange("p s c -> p (s c)"),
                        csb["c_iden"][:, :])
            A.copy(t[:, off:off + 128], tps[:, :])
        mtT2.append(t)

    # ---- per-sample sums via ones-matmuls: [1, 16] ----
    st_ps = ps_misc.tile([1, 16], F32, tag="ps")
    for ci, (off, k) in enumerate(CHUNKS):
        T.matmul(st_ps[:, :], csb["c_onescol"][0:k, 0:1], X[0:k, ci, :],
                 start=(ci == 0), stop=(ci == 4))
    sums = pp.tile([1, 16], F32, tag="sums")
    A.copy(sums[:, :], st_ps[:, :])
    musq = sp.tile([1, 8], F32, tag="musq")
    V.tensor_mul(musq[:], sums[:, 0:8], sums[:, 0:8])
    V.tensor_scalar_mul(musq[:], musq[:], -1.0 / (L - 1))
    var_ = sp.tile([1, 8], F32, tag="var_")
    V.tensor_add(var_[:], sums[:, 8:16], musq[:])
    stdv = sp.tile([1, 8], F32, tag="stdv")
    A.activation(stdv[:], var_[:], AF.Sqrt, scale=1.0 / (L - 2))
    V.tensor_scalar_add(stdv[:], stdv[:], 1e-6)
    inv = sp.tile([1, 8], F32, tag="inv")
    V.reciprocal(inv[:], stdv[:])
    ab_row = pp.tile([1, 8], F32, tag="ab_row")
    V.tensor_scalar_mul(ab_row[:], inv[:], 0.5)
    abP = ps_misc.tile([128, 1, 8], F32, tag="ps")
    T.matmul(abP[:, 0, :], csb["c_ones1"][:, :], ab_row[:, :], start=True, stop=True)

    # ---- score_col = a*sd + b*cos ; CLS sentinel; negated copy for ACT ----
    sc_t = sp.tile([128, 5, BC], F32, tag="sc_t")
    V.tensor_tensor(sc_t[:], X[:, :, 0:4],
                    abP[:, :, 0:4].broadcast_to([128, 5, 4]), op=EQ.mult)
    sc_u = sp.tile([128, 5, BC], F32, tag="sc_u")
    V.tensor_tensor(sc_u[:], X[:, :, 4:8],
                    abP[:, :, 4:8].broadcast_to([128, 5, 4]), op=EQ.mult)
    score_col = pp.tile([128, 5, BC], F32, tag="score_col")
    V.tensor_add(score_col[:], sc_t[:], sc_u[:])
    V.memset(score_col[0:1, 0, :], 1.0e30)
    neg_sc = pp.tile([128, 5, BC], F32, tag="neg_sc")
    V.tensor_scalar_mul(neg_sc[:], score_col[:], -1.0)

    if stage <= 2:
        return dump(2)

    # ---- score_row [BC, 640] via 5 transposes ----
    score_row = pp.tile([BC, LPAD], F32, tag="score_row")
    for ci, (off, k) in enumerate(CHUNKS):
        srp = ps_misc.tile([BC, 128], F32, tag="ps")
        T.transpose(srp[:, 0:k], score_col[0:k, ci, :], csb["c_iden"][0:k, 0:k])
        A.copy(score_row[:, off:off + k], srp[:, 0:k])

    # ---- rank: broadcast (PSUM) + fused compare-accum split V / ACT ----
    # V units accumulate #gt; ACT units accumulate sum(sign(s_j - s_i)),
    # whose mask threshold is 2*NSEL-576 instead of NSEL.
    rank = pp.tile([128, 5, BC], F32, tag="rank")
    G.memset(rank[:].rearrange("p c s -> p (c s)"), 1.0e9)
    for s in range(BC):
        bc_ps = ps_bcs.tile([128, LPAD], F32, tag="bcs")
        T.matmul(bc_ps[:, 0:512], csb["c_selbc"][:, s * 128:(s + 1) * 128],
                 score_row[:, 0:512], start=True, stop=True)
        T.matmul(bc_ps[:, 512:LPAD], csb["c_selbc"][:, s * 128:(s + 1) * 128],
                 score_row[:, 512:LPAD], start=True, stop=True)
        for ci, (off, k) in enumerate(CHUNKS):
            if RANK_ENG[(ci, s)] == 'V':
                g = sp.tile([128, LPAD], F32, tag="g")
                V.tensor_scalar(g[0:k, 0:L], bc_ps[0:k, 0:L],
                                score_col[0:k, ci, s:s + 1], 0.0,
                                op0=EQ.is_gt, op1=EQ.add,
                                accum_out=rank[0:k, ci, s:s + 1])
            else:
                ga = sp.tile([128, LPAD], F32, tag="ga")
                A.activation(ga[0:k, 0:L], bc_ps[0:k, 0:L], AF.Sign,
                             bias=neg_sc[0:k, ci, s:s + 1],
                             accum_out=rank[0:k, ci, s:s + 1])

    if stage <= 3:
        return dump(3)

    # ---- per-pair selection pipeline: pair p covers samples 2p, 2p+1 ----
    # msk/cums/targets/sim/C/big-MMs for pair 0 run while pair 1's rank
    # units are still executing.
    msk_f = pp.tile([128, 5, BC], F32, tag="msk_f")
    msk_b = pp.tile([128, 5, BC, 1], BF16, tag="msk_b")
    notm = pp.tile([128, 5, BC, 1], F32, tag="notm")
    cums = pp.tile([128, 5, BC, 1], F32, tag="cums")
    G.memset(cums[:].rearrange("p c s o -> p (c s o)"), 0.0)
    pn = pp.tile([128, 5, BC, 1], F32, tag="pn")
    itgt = pp.tile([128, 5, BC, CTX], F32, tag="itgt")
    itgt_w = pp.tile([128, 5, BC, CTX], F32, tag="itgt_w")
    ismrg = pp.tile([128, 5, BC, 1], F32, tag="ismrg")
    G.memset(ismrg[:].rearrange("p c s o -> p (c s o)"), 0.0)
    eqm = pp.tile([128, 5, BC, CTX], F32, tag="eqm")
    crb = pp.tile([128, BC, CTX], F32, tag="crb")
    cts = pp.tile([128, 5, BC, 80], BF16, tag="cts")
    thrV = float(NSEL)
    thrA = float(2 * NSEL - (L - 1))
    zblk = sp.tile([128, 2 * CTX], F32, tag="zblk")
    V.memset(zblk[:], 0.0)

    for p in range(2):
        sl = slice(2 * p, 2 * p + 2)
        # mask / not-mask per engine-encoding block (V: #gt, A: sum-of-sign)
        for (c0, c1, thr) in _msk_blocks(p):
            for ci in range(c0, c1):
                V.tensor_scalar(msk_f[:, ci, sl], rank[:, ci, sl],
                                thr, None, op0=EQ.is_lt)
                V.tensor_scalar(msk_b[:, ci, sl, 0], rank[:, ci, sl],
                                thr, None, op0=EQ.is_lt)
                V.tensor_scalar(notm[:, ci, sl, 0], rank[:, ci, sl],
                                thr, None, op0=EQ.is_ge)
        # cums (upper-triangular ones matmuls, bf16)
        for cm in range(5):
            kcm = CHUNKS[cm][1]
            cps = ps_misc.tile([128, 2], F32, tag="ps")
            for ck in range(cm + 1):
                lhs = csb["c_utb"] if ck == cm else csb["c_onesb"]
                kk = CHUNKS[ck][1]
                T.matmul(cps[0:kcm, :], lhs[0:kk, 0:kcm], msk_b[0:kk, ck, sl, 0],
                         start=(ck == 0), stop=(ck == cm))
            A.copy(cums[0:kcm, cm, sl, 0], cps[0:kcm, :])
        V.tensor_tensor(pn[:, :, sl, 0], cums[:, :, sl, 0],
                        csb["c_iotaI"][:].broadcast_to([128, 5, 2]), op=EQ.subtract)
        # targets / merge membership
        for ci, (off, k) in enumerate(CHUNKS):
            V.tensor_tensor(itgt[0:k, ci, sl],
                            csb["c_iota52"][0:k].broadcast_to([k, 2, CTX]),
                            pn[0:k, ci, sl].broadcast_to([k, 2, CTX]), op=EQ.is_equal)
            V.tensor_tensor(itgt[0:k, ci, sl], itgt[0:k, ci, sl],
                            notm[0:k, ci, sl].broadcast_to([k, 2, CTX]), op=EQ.mult)
            V.tensor_tensor(itgt_w[0:k, ci, sl], itgt[0:k, ci, sl],
                            nrm[0:k, ci, sl].broadcast_to([k, 2, CTX]), op=EQ.mult)
            tany = sp.tile([128, 2], F32, tag="tany")
            V.tensor_reduce(tany[0:k], itgt[0:k, ci, sl], axis=AX.X, op=EQ.add)
            omt = sp.tile([128, 2], F32, tag="omt")
            V.tensor_scalar(omt[0:k], tany[0:k], -1.0, 1.0, op0=EQ.mult, op1=EQ.add)
            V.tensor_mul(ismrg[0:k, ci, sl, 0], notm[0:k, ci, sl, 0], omt[0:k])
        # Tn per sample -> block-diagonal tnD
        td = pp.tile([128, 2, CTX], F32, tag=f"tnD{p}")
        A.copy(td[:].rearrange("p s c -> p (s c)"), zblk[:])
        for half in range(2):
            s = 2 * p + half
            tn_ps = ps_misc.tile([CK, CTX], F32, tag="ps")
            for ci, (off, k) in enumerate(CHUNKS):
                T.matmul(tn_ps[:, :], mt[0:k, ci, s, :], itgt_w[0:k, ci, s, :],
                         start=(ci == 0), stop=(ci == 4))
            A.copy(td[64 * half:64 * half + 64, half, :], tn_ps[:, :])
        # sim (batched over the pair), rmx, eqm
        for ci, (off, k) in enumerate(CHUNKS):
            sim_ps = ps_misc.tile([128, 2, CTX], F32, tag="ps")
            T.matmul(sim_ps[:].rearrange("p s c -> p (s c)"),
                     mtT2[p][:, off:off + 128], td[:].rearrange("p s c -> p (s c)"),
                     start=True, stop=True)
            rmx = sp.tile([128, 2, 1], F32, tag="rmx")
            V.tensor_reduce(rmx[0:k, :, 0], sim_ps[0:k], axis=AX.X, op=EQ.max)
            V.tensor_tensor(eqm[0:k, ci, sl], sim_ps[0:k],
                            rmx[0:k].broadcast_to([k, 2, CTX]), op=EQ.is_ge)
            V.tensor_tensor(eqm[0:k, ci, sl], eqm[0:k, ci, sl],
                            ismrg[0:k, ci, sl].broadcast_to([k, 2, CTX]),
                            op=EQ.mult)
        # counts -> 1/cnt row -> partition broadcast
        cnt_ps = ps_misc.tile([1, 2 * CTX], F32, tag="ps")
        for ci, (off, k) in enumerate(CHUNKS):
            T.matmul(cnt_ps[:, :], csb["c_onescol"][0:k, 0:1],
                     eqm[0:k, ci, sl].rearrange("p s c -> p (s c)"),
                     start=(ci == 0), stop=(ci == 4))
        cmax = sp.tile([1, 2 * CTX], F32, tag="cmax")
        V.tensor_scalar_max(cmax[:], cnt_ps[:, :], 1.0)
        crec_row = sp.tile([1, 2 * CTX], F32, tag="crec_row")
        V.reciprocal(crec_row[:], cmax[:])
        crb_ps = ps_misc.tile([128, 2 * CTX], F32, tag="ps")
        T.matmul(crb_ps[:, :], csb["c_ones1"][:, :], crec_row[:, :],
                 start=True, stop=True)
        A.copy(crb[:, sl].rearrange("p s c -> p (s c)"), crb_ps[:, :])
        # C build (bf16)
        for ci, (off, k) in enumerate(CHUNKS):
            dom = sp.tile([128, 2, NSEL], F32, tag="dom")
            V.tensor_tensor(dom[0:k], csb["c_iota55"][0:k].broadcast_to([k, 2, NSEL]),
                            cums[0:k, ci, sl].broadcast_to([k, 2, NSEL]),
                            op=EQ.is_equal)
            V.tensor_tensor(cts[0:k, ci, sl, 0:NSEL], dom[0:k],
                            msk_f[0:k, ci, sl].rearrange("p s -> p s ()")
                            .broadcast_to([k, 2, NSEL]), op=EQ.mult)
            wct = sp.tile([128, 2, CTX], F32, tag="wct")
            V.tensor_mul(wct[0:k], eqm[0:k, ci, sl], crb[0:k, sl])
            V.tensor_add(cts[0:k, ci, sl, NSEL:OUT_T], wct[0:k], itgt[0:k, ci, sl])
        # big matmuls (bf16) + ACT copy + out DMA
        for half in range(2):
            s = 2 * p + half
            for n2 in range(2):
                po = ps_out.tile([OUT_T, 512], F32, tag="po")
                for ci, (off, k) in enumerate(CHUNKS):
                    T.matmul(po[:, :], cts[0:k, ci, s, 0:OUT_T],
                             hid[s][ci][0:k, n2 * 512:(n2 + 1) * 512],
                             start=(ci == 0), stop=(ci == 4))
                ob = sp.tile([OUT_T, 512], F32, tag="ob", bufs=3)
                if n2 == 0:
                    A.copy(ob[:, :], po[:, :])
                else:
                    V.tensor_scalar_add(ob[:, :], po[:, :], 0.0)
                DMA.dma_start(out_d[s, :, n2 * 512:(n2 + 1) * 512], ob[:, :])


def _msk_blocks(p):
    """Chunk-range blocks of the (ci, s-pair) grid sharing one rank encoding."""
    if p == 0:   # samples 0,1: V for ci<=2, A for ci>=3
        return [(0, 3, float(NSEL)), (3, 5, float(2 * NSEL - (L - 1)))]
    else:        # samples 2,3: V for ci<2, A for ci>=2
        return [(0, 2, float(NSEL)), (2, 5, float(2 * NSEL - (L - 1)))]


_NC = None


def _get_nc():
    global _NC
    if _NC is None:
        _NC = build_nc()
    return _NC


def shard_inputs(attn_weights, hidden_states, metric, text_emb):
    """Host-side shard: slice CLS attention row, transpose to token-major
    column layout, cast hidden to bf16, split batch across cores."""
    B = attn_weights.shape[0]
    per = B // N_CORES
    attn_row = np.ascontiguousarray(attn_weights[:, :, 0, :], dtype=np.float32)
    h_b = np.asarray(hidden_states, np.float32).astype(NPBF16)
    met = np.asarray(metric, np.float32)
    consts = _consts()
    in_maps = []
    for c in range(N_CORES):
        sl = slice(c * per, (c + 1) * per)
        at = attn_row[sl].transpose(2, 0, 1)                   # [577, 4, 16]
        atp = np.zeros((LPAD, per, NH), np.float32)
        atp[:L] = at
        atT = np.ascontiguousarray(
            atp.reshape(5, 128, per * NH).transpose(1, 0, 2))  # [128, 5, 64]
        mtc = met[sl].transpose(1, 0, 2)                       # [577, 4, 64]
        mtp = np.zeros((LPAD, per, CK), np.float32)
        mtp[:L] = mtc
        mtT = np.ascontiguousarray(
            mtp.reshape(5, 128, per, CK).transpose(1, 0, 2, 3))
        m = {
            "attnT": atT,
            "metricT": mtT,
            "text": np.ascontiguousarray(text_emb[sl]).astype(np.float32),
            "hidb": np.ascontiguousarray(h_b[sl]),
        }
        m.update(consts)
        in_maps.append(m)
    return in_maps


def kernel(attn_weights, hidden_states, metric, text_emb):
    nc = _get_nc()
    in_maps = shard_inputs(attn_weights, hidden_states, metric, text_emb)
    res = run_bass_kernel_spmd(nc, in_maps, core_ids=list(range(N_CORES)))
    out = np.concatenate([r["out"] for r in res.results], axis=0)
    return out.astype(np.float32)
